# revision 2
# baseline (speedup 1.0000x reference)
"""Trainium2 Bass kernel for LocalSelfAttention2d.

Full inputs in, full outputs out. Data-parallel over batch B=16 across 8
NeuronCores (2 images per core). Weights/position table replicated.

v2 redesign vs v1 (all per core, per image):
  - x arrives bf16 (host-cast), DMA'd raster and window-major-scattered on
    GPSIMD.
  - q/k proj into per-(mc, window-row) SBUF tiles [128, 1024] bf16,
    evacuated on the scalar engine (ACT).
  - scores per window-PAIR: one [128, 1024] PSUM tile (2 banks), rows =
    j over both windows, cols = (hg, r, i-both-windows). 8 matmuls
    (M=128, N=128, K=32 at row group r*32) — cross-window quadrants hold
    bounded garbage.
  - exp over the whole [128, 1024] on ACT, then one DVE bf16 multiply by
    exp(bias) with zeros in cross-window quadrants: masks garbage AND
    applies the relative-position bias (exp(s+b) = exp(s)*exp(b)).
  - o + denominator per 2-pair block and head-group: one PSUM bank
    [128, 512]: cols 0:256 = oT (rows r*32+d), cols 256:512 = denominator
    replicated over 32 rows via ones-matmul (K=128 contracts the masked
    attn, so cross-window terms vanish). reciprocal + one fused
    normalize-evacuate TT into window-major o_all.
  - out proj + per-partition bias via ACT Identity(bias), DMA out fp32.
"""
import numpy as np

B, C, H, W = 16, 256, 64, 64
P, HEADS, D = 8, 8, 32
NCORES = 8
B_LOC = B // NCORES  # 2
HW = H * W  # 4096
NW = 8  # windows per row/col

_CACHE = {}


def _rel_bias_np(position):
    coords = np.stack(
        np.meshgrid(np.arange(P), np.arange(P), indexing="ij"), -1
    ).reshape(P * P, 2)
    rel = coords[None, :, :] - coords[:, None, :] + P
    return position[:, rel[..., 0], rel[..., 1]]  # [heads, 64, 64] (h, i, j)


def _wm_scatter(t, wr):
    """Window-major dest AP for raster source of window-row wr."""
    v = t.rearrange("p (wh ww ph pw) -> p wh ph ww pw", wh=NW, ww=NW, ph=P, pw=P)
    return v[:, wr]


def _raster_scatter(t):
    """Raster dest AP [128, ww, ph, pw] for window-major-ordered source."""
    return t.rearrange("p (ph ww pw) -> p ww ph pw", ph=P, ww=NW, pw=P)


def _build():
    import concourse.bass as bass  # noqa: F401
    import concourse.tile as tile
    from concourse import bacc, mybir

    f32 = mybir.dt.float32
    bf16 = mybir.dt.bfloat16
    MULT = mybir.AluOpType.mult
    EXP = mybir.ActivationFunctionType.Exp
    IDENT = mybir.ActivationFunctionType.Identity

    nc = bacc.Bacc("TRN2", target_bir_lowering=False, debug=False,
                   num_devices=NCORES)

    x_d = nc.dram_tensor("x_sh", [B_LOC, C, HW], f32, kind="ExternalInput").ap()
    wqk_d = nc.dram_tensor("w_qkT", [C, 512], bf16, kind="ExternalInput").ap()
    wv_d = nc.dram_tensor("w_vT", [C, 256], bf16, kind="ExternalInput").ap()
    wo_d = nc.dram_tensor("w_outT", [C, 256], bf16, kind="ExternalInput").ap()
    eb_d = nc.dram_tensor("eb_full", [128, 2048], bf16, kind="ExternalInput").ap()
    ones_d = nc.dram_tensor("ones_c", [128, 32], bf16, kind="ExternalInput").ap()
    bout_d = nc.dram_tensor("b_out2", [2, 128, 1], f32, kind="ExternalInput").ap()
    y_d = nc.dram_tensor("y_sh", [B_LOC, C, HW], f32, kind="ExternalOutput").ap()

    with tile.TileContext(nc) as tc:
        with (
            tc.tile_pool(name="const", bufs=1) as constp,
            tc.tile_pool(name="xin", bufs=4) as xinp,
            tc.tile_pool(name="xbf", bufs=2) as xbfp,
            tc.tile_pool(name="qkbf", bufs=2) as qkp,
            tc.tile_pool(name="vtbf", bufs=2) as vtp,
            tc.tile_pool(name="att", bufs=2) as attp,
            tc.tile_pool(name="rcp", bufs=4) as rcpp,
            tc.tile_pool(name="oall", bufs=1) as oallp,
            tc.tile_pool(name="ysb", bufs=4) as ysbp,
            tc.tile_pool(name="psum", bufs=1, space="PSUM") as psp,
        ):
            # ---- constants: only wqk is needed immediately; the rest are
            # DMA'd after the first image's x loads are queued ----
            wqk_sb = []
            wv_sb = []
            wo_sb = []
            bout_sb = []
            for kc in range(2):
                t = constp.tile([128, 512], bf16, tag=f"wqk{kc}", name=f"wqk{kc}")
                wqk_sb.append(t)
                t = constp.tile([128, 256], bf16, tag=f"wv{kc}", name=f"wv{kc}")
                wv_sb.append(t)
                t = constp.tile([128, 256], bf16, tag=f"wo{kc}", name=f"wo{kc}")
                wo_sb.append(t)
                t = constp.tile([128, 1], f32, tag=f"bo{kc}", name=f"bo{kc}")
                bout_sb.append(t)
            eb_sb = constp.tile([128, 2048], bf16, tag="eb", name="eb")
            ones_sb = constp.tile([128, 32], bf16, tag="ones", name="ones")

            def load_late_consts():
                for kc in range(2):
                    nc.sync.dma_start(
                        wv_sb[kc][:], wv_d[kc * 128 : (kc + 1) * 128, :])
                    nc.sync.dma_start(
                        wo_sb[kc][:], wo_d[kc * 128 : (kc + 1) * 128, :])
                    nc.sync.dma_start(bout_sb[kc][:], bout_d[kc])
                nc.sync.dma_start(eb_sb[:], eb_d[:])
                nc.sync.dma_start(ones_sb[:], ones_d[:])

            def load_image(b):
                # phase A: load x, window-major scatter on GPSIMD.
                # per-(cc, window-row) tiles so downstream matmuls can start
                # as soon as the first window-row lands.
                xw = {}
                for t4 in range(4):
                    for cc in range(2):
                        xi = xinp.tile([128, 1024], f32, tag="xin", name="xin")
                        (nc.sync if cc == 0 else nc.gpsimd).dma_start(
                            xi[:],
                            x_d[b, cc * 128 : (cc + 1) * 128,
                                t4 * 1024 : (t4 + 1) * 1024],
                        )
                        for u in range(2):
                            wr = t4 * 2 + u
                            xt = xbfp.tile([128, 512], bf16,
                                           tag=f"xw{cc}_{wr}", name="xwt")
                            src = xi[:, u * 512 : (u + 1) * 512].rearrange(
                                "p (ph ww pw) -> p ph ww pw", ph=P, ww=NW, pw=P
                            )
                            dst = xt.rearrange(
                                "p (ww ph pw) -> p ph ww pw", ww=NW, ph=P, pw=P
                            )
                            nc.gpsimd.tensor_copy(dst, src)
                            xw[(cc, wr)] = xt
                    if b == 0 and t4 == 0:
                        # x wr0 is the critical path; wqk slots in behind it
                        for kc in range(2):
                            nc.sync.dma_start(
                                wqk_sb[kc][:],
                                wqk_d[kc * 128 : (kc + 1) * 128, :])
                if b == 0:
                    load_late_consts()
                return xw

            xw_next = load_image(0)
            for b in range(B_LOC):
                xw = xw_next

                # ---- phase B: q/k proj -> qk_t[(mc, nt2)] [128, 1024] bf16 ----
                # evacuation split between ACT and DVE halves to keep up
                # with PE's fill rate.
                qk_t = {}
                for nt2 in range(4):
                    for mc in range(4):
                        idx = nt2 * 4 + mc
                        ps = psp.tile([128, 1024], f32,
                                      tag=("scA" if idx % 2 == 0 else "scB"),
                                      bufs=1, name="psc")
                        for half in range(2):
                            for kc in range(2):
                                nc.tensor.matmul(
                                    ps[:, half * 512 : (half + 1) * 512],
                                    lhsT=wqk_sb[kc][:, mc * 128 : (mc + 1) * 128],
                                    rhs=xw[(kc, nt2 * 2 + half)][:],
                                    start=(kc == 0),
                                    stop=(kc == 1),
                                )
                        qt = qkp.tile([128, 1024], bf16, tag=f"qk{mc}_{nt2}",
                                      name="qkt")
                        nc.scalar.copy(qt[:, 0:512], ps[:, 0:512])
                        nc.vector.tensor_copy(qt[:, 512:1024], ps[:, 512:1024])
                        qk_t[(mc, nt2)] = qt

                # ---- phase C: v proj -> vt [128, 8192] bf16 (p, h, d) ----
                vt = vtp.tile([128, 32 * 256], bf16, tag="vt", name="vt")
                for p2 in range(16):
                    ps = psp.tile([128, 512], f32, tag="od", bufs=4, name="pod")
                    for half in range(2):
                        p = p2 * 2 + half
                        for kc in range(2):
                            nc.tensor.matmul(
                                ps[:, half * 256 : (half + 1) * 256],
                                lhsT=xw[(kc, p // 4)][:, (p % 4) * 128 :
                                                      (p % 4 + 1) * 128],
                                rhs=wv_sb[kc][:],
                                start=(kc == 0),
                                stop=(kc == 1),
                            )
                    if p2 % 2 == 0:
                        nc.vector.tensor_copy(
                            vt[:, p2 * 512 : (p2 + 1) * 512], ps[:])
                    else:
                        nc.scalar.copy(
                            vt[:, p2 * 512 : (p2 + 1) * 512], ps[:])

                # ---- phase D: attention per window pair ----
                oa = {}
                for nt in range(8):
                    for hg in range(2):
                        oa[(hg, nt)] = oallp.tile(
                            [128, 512], bf16, tag=f"oall{hg}_{nt}", name="oat"
                        )

                def oden_block(bk, at2):
                    # o + denominator for pairs (2bk, 2bk+1) from masked attn
                    # at2 cols = (r, pp, hg, i2w)
                    p0 = 2 * bk
                    for hg in range(2):
                        od = psp.tile([128, 512], f32, tag="od", bufs=4,
                                      name="pod")
                        for q in range(2):
                            for r in range(4):
                                h = 4 * hg + r
                                nc.tensor.matmul(
                                    od[r * 32 : (r + 1) * 32,
                                       q * 128 : (q + 1) * 128],
                                    lhsT=vt[:, (p0 + q) * 256 + h * 32 :
                                            (p0 + q) * 256 + (h + 1) * 32],
                                    rhs=at2[:, r * 512 + q * 256 + hg * 128 :
                                            r * 512 + q * 256 + (hg + 1) * 128],
                                    start=True, stop=True,
                                    tile_position=(0, r * 32),
                                )
                        at4 = at2.rearrange("p (r q x) -> p r q x",
                                            r=4, q=2, x=256)
                        for r in range(4):
                            nc.tensor.matmul(
                                od[r * 32 : (r + 1) * 32, 256:512],
                                lhsT=ones_sb[:],
                                rhs=at4[:, r, :, hg * 128 : (hg + 1) * 128],
                                start=True, stop=True,
                                tile_position=(0, r * 32),
                            )
                        rc = rcpp.tile([128, 256], f32, tag="rc", name="rc")
                        nc.vector.reciprocal(rc[:], od[:, 256:512])
                        nc.vector.tensor_tensor(
                            oa[(hg, bk // 2)][:, (bk % 2) * 256 :
                                              (bk % 2 + 1) * 256],
                            od[:, 0:256], rc[:], MULT,
                        )

                def out_proj(nt):
                    # output projection for spatial block nt (512 cols)
                    for mc in range(2):
                        ps = psp.tile([128, 512], f32, tag="od", bufs=4,
                                      name="pod")
                        for kc in range(2):
                            nc.tensor.matmul(
                                ps[:],
                                lhsT=wo_sb[kc][:, mc * 128 : (mc + 1) * 128],
                                rhs=oa[(kc, nt)][:],
                                start=(kc == 0),
                                stop=(kc == 1),
                            )
                        yt = ysbp.tile([128, 512], f32, tag="ysb", name="ysb")
                        psv = ps.rearrange(
                            "p (ww ph pw) -> p ww ph pw", ww=NW, ph=P, pw=P
                        )
                        nc.vector.tensor_scalar_add(
                            _raster_scatter(yt), psv, bout_sb[mc][:],
                        )
                        nc.sync.dma_start(
                            y_d[b, mc * 128 : (mc + 1) * 128,
                                nt * 512 : (nt + 1) * 512],
                            yt[:],
                        )

                at_prev = None
                for bk in range(16):
                    at2 = attp.tile([128, 2048], bf16, tag="at2", name="at2")
                    # scores for both pairs of the block: bank = head strip r
                    # (single row-group tile_position per PSUM bank), cols
                    # within a bank = (pp, hg, i2w)
                    scA = psp.tile([128, 1024], f32, tag="scA", bufs=1,
                                   name="psc")
                    scB = psp.tile([128, 1024], f32, tag="scB", bufs=1,
                                   name="psc")
                    for pp in range(2):
                        p = 2 * bk + pp
                        nt2, col = p // 8, (p % 8) * 128
                        for hg in range(2):
                            for r in range(4):
                                sc = scA if r < 2 else scB
                                nc.tensor.matmul(
                                    sc[:, (r % 2) * 512 + pp * 256 + hg * 128 :
                                       (r % 2) * 512 + pp * 256 + (hg + 1) * 128],
                                    lhsT=qk_t[(2 + hg, nt2)][r * 32 : (r + 1) * 32,
                                                             col : col + 128],
                                    rhs=qk_t[(hg, nt2)][r * 32 : (r + 1) * 32,
                                                        col : col + 128],
                                    start=True, stop=True,
                                    tile_position=(r * 32, 0),
                                )
                    for r in range(4):
                        sc = scA if r < 2 else scB
                        nc.scalar.activation(
                            at2[:, r * 512 : (r + 1) * 512],
                            sc[:, (r % 2) * 512 : (r % 2 + 1) * 512], EXP
                        )
                    # bf16 multiply masks cross-window quadrants and applies
                    # exp(bias)
                    nc.vector.tensor_tensor(at2[:], at2[:], eb_sb[:], MULT)
                    if at_prev is not None:
                        oden_block(bk - 1, at_prev)
                        if bk % 2 == 0:
                            out_proj((bk - 2) // 2)
                    at_prev = at2
                    if bk == 10 and b + 1 < B_LOC:
                        xw_next = load_image(b + 1)
                oden_block(15, at_prev)
                out_proj(7)

    nc.compile()
    return nc


def _prep_consts(w_proj, position, w_out, b_out):
    import ml_dtypes
    bf16 = ml_dtypes.bfloat16
    scale = 1.0 / np.sqrt(np.float32(D))
    w_qkT = np.ascontiguousarray(w_proj[:512].T).astype(np.float32)
    w_qkT[:, :256] *= scale
    w_qkT = w_qkT.astype(bf16)
    w_vT = np.ascontiguousarray(w_proj[512:].T).astype(bf16)
    w_outT = np.ascontiguousarray(w_out.T).astype(bf16)
    bias = _rel_bias_np(position.astype(np.float32))  # [h, i(query), j(key)]
    # eb_full[wj*64+j, r*512 + pp*256 + hg*128 + wi*64 + i] =
    #   exp(bias[4hg+r][i, j]) if wi == wj else 0   (replicated over pp)
    eb = np.zeros((128, 2048), np.float32)
    for r in range(4):
        for pp in range(2):
            for hg in range(2):
                ebT = np.exp(bias[4 * hg + r].T)  # [j, i]
                base = r * 512 + pp * 256 + hg * 128
                for wj in range(2):
                    eb[wj * 64 : (wj + 1) * 64,
                       base + wj * 64 : base + (wj + 1) * 64] = ebT
    eb_full = eb.astype(bf16)
    ones_c = np.ones((128, 32), bf16)
    b_out2 = np.ascontiguousarray(
        b_out.astype(np.float32).reshape(2, 128, 1)
    )
    return {
        "w_qkT": w_qkT,
        "w_vT": w_vT,
        "w_outT": w_outT,
        "eb_full": eb_full,
        "ones_c": ones_c,
        "b_out2": b_out2,
    }


def kernel(x, w_proj, position, w_out, b_out):
    import ml_dtypes
    from concourse.bass_utils import run_bass_kernel_spmd

    if "nc" not in _CACHE:
        _CACHE["nc"] = _build()
    nc = _CACHE["nc"]

    consts = _prep_consts(w_proj, position, w_out, b_out)
    x = np.ascontiguousarray(np.asarray(x, np.float32).reshape(B, C, HW))
    in_maps = []
    for i in range(NCORES):
        m = dict(consts)
        m["x_sh"] = np.ascontiguousarray(x[i * B_LOC : (i + 1) * B_LOC])
        in_maps.append(m)

    res = run_bass_kernel_spmd(nc, in_maps, core_ids=list(range(NCORES)))
    out = np.concatenate([res.results[i]["y_sh"] for i in range(NCORES)], axis=0)
    return out.reshape(B, C, H, W)


# revision 5
# speedup vs baseline: 1.1621x; 1.1621x over previous
"""Trainium2 Bass kernel for LocalSelfAttention2d.

Full inputs in, full outputs out. Data-parallel over batch B=16 across 8
NeuronCores (2 images per core). Weights/position table replicated.

v2 redesign vs v1 (all per core, per image):
  - x arrives bf16 (host-cast), DMA'd raster and window-major-scattered on
    GPSIMD.
  - q/k proj into per-(mc, window-row) SBUF tiles [128, 1024] bf16,
    evacuated on the scalar engine (ACT).
  - scores per window-PAIR: one [128, 1024] PSUM tile (2 banks), rows =
    j over both windows, cols = (hg, r, i-both-windows). 8 matmuls
    (M=128, N=128, K=32 at row group r*32) — cross-window quadrants hold
    bounded garbage.
  - exp over the whole [128, 1024] on ACT, then one DVE bf16 multiply by
    exp(bias) with zeros in cross-window quadrants: masks garbage AND
    applies the relative-position bias (exp(s+b) = exp(s)*exp(b)).
  - o + denominator per 2-pair block and head-group: one PSUM bank
    [128, 512]: cols 0:256 = oT (rows r*32+d), cols 256:512 = denominator
    replicated over 32 rows via ones-matmul (K=128 contracts the masked
    attn, so cross-window terms vanish). reciprocal + one fused
    normalize-evacuate TT into window-major o_all.
  - out proj + per-partition bias via ACT Identity(bias), DMA out fp32.
"""
import numpy as np

B, C, H, W = 16, 256, 64, 64
P, HEADS, D = 8, 8, 32
NCORES = 8
B_LOC = B // NCORES  # 2
HW = H * W  # 4096
NW = 8  # windows per row/col

_CACHE = {}


def _rel_bias_np(position):
    coords = np.stack(
        np.meshgrid(np.arange(P), np.arange(P), indexing="ij"), -1
    ).reshape(P * P, 2)
    rel = coords[None, :, :] - coords[:, None, :] + P
    return position[:, rel[..., 0], rel[..., 1]]  # [heads, 64, 64] (h, i, j)


def _wm_scatter(t, wr):
    """Window-major dest AP for raster source of window-row wr."""
    v = t.rearrange("p (wh ww ph pw) -> p wh ph ww pw", wh=NW, ww=NW, ph=P, pw=P)
    return v[:, wr]


def _raster_scatter(t):
    """Raster dest AP [128, ww, ph, pw] for window-major-ordered source."""
    return t.rearrange("p (ph ww pw) -> p ww ph pw", ph=P, ww=NW, pw=P)


def _build():
    import concourse.bass as bass  # noqa: F401
    import concourse.tile as tile
    from concourse import bacc, mybir

    f32 = mybir.dt.float32
    bf16 = mybir.dt.bfloat16
    MULT = mybir.AluOpType.mult
    EXP = mybir.ActivationFunctionType.Exp
    IDENT = mybir.ActivationFunctionType.Identity

    nc = bacc.Bacc("TRN2", target_bir_lowering=False, debug=False,
                   num_devices=NCORES)

    x_d = nc.dram_tensor("x_sh", [B_LOC, C, HW], bf16, kind="ExternalInput").ap()
    wqk_d = nc.dram_tensor("w_qkT", [C, 512], bf16, kind="ExternalInput").ap()
    wv_d = nc.dram_tensor("w_vT", [C, 256], bf16, kind="ExternalInput").ap()
    wo_d = nc.dram_tensor("w_outT", [C, 256], bf16, kind="ExternalInput").ap()
    eb_d = nc.dram_tensor("eb_full", [128, 2048], bf16, kind="ExternalInput").ap()
    ones_d = nc.dram_tensor("ones_c", [128, 32], bf16, kind="ExternalInput").ap()
    bout_d = nc.dram_tensor("b_out2", [2, 128, 1], f32, kind="ExternalInput").ap()
    y_d = nc.dram_tensor("y_sh", [B_LOC, C, HW], f32, kind="ExternalOutput").ap()

    with tile.TileContext(nc) as tc:
        with (
            tc.tile_pool(name="const", bufs=1) as constp,
            tc.tile_pool(name="xin", bufs=4) as xinp,
            tc.tile_pool(name="xbf", bufs=2) as xbfp,
            tc.tile_pool(name="qkbf", bufs=2) as qkp,
            tc.tile_pool(name="vtbf", bufs=2) as vtp,
            tc.tile_pool(name="att", bufs=2) as attp,
            tc.tile_pool(name="rcp", bufs=4) as rcpp,
            tc.tile_pool(name="oall", bufs=1) as oallp,
            tc.tile_pool(name="ysb", bufs=4) as ysbp,
            tc.tile_pool(name="psum", bufs=1, space="PSUM") as psp,
        ):
            # ---- constants: only wqk is needed immediately; the rest are
            # DMA'd after the first image's x loads are queued ----
            wqk_sb = []
            wv_sb = []
            wo_sb = []
            bout_sb = []
            for kc in range(2):
                t = constp.tile([128, 512], bf16, tag=f"wqk{kc}", name=f"wqk{kc}")
                wqk_sb.append(t)
                t = constp.tile([128, 256], bf16, tag=f"wv{kc}", name=f"wv{kc}")
                wv_sb.append(t)
                t = constp.tile([128, 256], bf16, tag=f"wo{kc}", name=f"wo{kc}")
                wo_sb.append(t)
                t = constp.tile([128, 1], f32, tag=f"bo{kc}", name=f"bo{kc}")
                bout_sb.append(t)
            eb_sb = constp.tile([128, 2048], bf16, tag="eb", name="eb")
            ones_sb = constp.tile([128, 32], bf16, tag="ones", name="ones")

            def load_late_consts():
                for kc in range(2):
                    nc.sync.dma_start(
                        wv_sb[kc][:], wv_d[kc * 128 : (kc + 1) * 128, :])
                    nc.sync.dma_start(
                        wo_sb[kc][:], wo_d[kc * 128 : (kc + 1) * 128, :])
                    nc.sync.dma_start(bout_sb[kc][:], bout_d[kc])
                nc.sync.dma_start(eb_sb[:], eb_d[:])
                nc.sync.dma_start(ones_sb[:], ones_d[:])

            def load_image(b):
                # phase A: load x, window-major scatter on GPSIMD.
                # per-(cc, window-row) tiles so downstream matmuls can start
                # as soon as the first window-row lands.
                xw = {}
                for t4 in range(4):
                    for cc in range(2):
                        xi = xinp.tile([128, 1024], bf16, tag="xin", name="xin")
                        (nc.sync if cc == 0 else nc.gpsimd).dma_start(
                            xi[:],
                            x_d[b, cc * 128 : (cc + 1) * 128,
                                t4 * 1024 : (t4 + 1) * 1024],
                        )
                        for u in range(2):
                            wr = t4 * 2 + u
                            xt = xbfp.tile([128, 512], bf16,
                                           tag=f"xw{cc}_{wr}", name="xwt")
                            src = xi[:, u * 512 : (u + 1) * 512].rearrange(
                                "p (ph ww pw) -> p ph ww pw", ph=P, ww=NW, pw=P
                            )
                            dst = xt.rearrange(
                                "p (ww ph pw) -> p ph ww pw", ww=NW, ph=P, pw=P
                            )
                            nc.gpsimd.tensor_copy(dst, src)
                            xw[(cc, wr)] = xt
                    if b == 0 and t4 == 0:
                        # x wr0 is the critical path; wqk slots in behind it
                        for kc in range(2):
                            nc.sync.dma_start(
                                wqk_sb[kc][:],
                                wqk_d[kc * 128 : (kc + 1) * 128, :])
                if b == 0:
                    load_late_consts()
                return xw

            xw_next = load_image(0)
            for b in range(B_LOC):
                xw = xw_next

                # ---- phase B: q/k proj -> qk_t[(mc, nt2)] [128, 1024] bf16 ----
                # evacuation split between ACT and DVE halves to keep up
                # with PE's fill rate.
                qk_t = {}
                for nt2 in range(4):
                    for mc in range(4):
                        idx = nt2 * 4 + mc
                        ps = psp.tile([128, 1024], f32,
                                      tag=("scA" if idx % 2 == 0 else "scB"),
                                      bufs=1, name="psc")
                        for half in range(2):
                            for kc in range(2):
                                nc.tensor.matmul(
                                    ps[:, half * 512 : (half + 1) * 512],
                                    lhsT=wqk_sb[kc][:, mc * 128 : (mc + 1) * 128],
                                    rhs=xw[(kc, nt2 * 2 + half)][:],
                                    start=(kc == 0),
                                    stop=(kc == 1),
                                )
                        qt = qkp.tile([128, 1024], bf16, tag=f"qk{mc}_{nt2}",
                                      name="qkt")
                        nc.scalar.copy(qt[:, 0:512], ps[:, 0:512])
                        nc.vector.tensor_copy(qt[:, 512:1024], ps[:, 512:1024])
                        qk_t[(mc, nt2)] = qt

                # ---- phase C: v proj -> vt [128, 8192] bf16 (p, h, d) ----
                vt = vtp.tile([128, 32 * 256], bf16, tag="vt", name="vt")
                for p2 in range(16):
                    ps = psp.tile([128, 512], f32, tag="od", bufs=4, name="pod")
                    for half in range(2):
                        p = p2 * 2 + half
                        for kc in range(2):
                            nc.tensor.matmul(
                                ps[:, half * 256 : (half + 1) * 256],
                                lhsT=xw[(kc, p // 4)][:, (p % 4) * 128 :
                                                      (p % 4 + 1) * 128],
                                rhs=wv_sb[kc][:],
                                start=(kc == 0),
                                stop=(kc == 1),
                            )
                    if p2 % 2 == 0:
                        nc.vector.tensor_copy(
                            vt[:, p2 * 512 : (p2 + 1) * 512], ps[:])
                    else:
                        nc.scalar.copy(
                            vt[:, p2 * 512 : (p2 + 1) * 512], ps[:])

                # ---- phase D: attention per window pair ----
                oa = {}
                for nt in range(8):
                    for hg in range(2):
                        oa[(hg, nt)] = oallp.tile(
                            [128, 512], bf16, tag=f"oall{hg}_{nt}", name="oat"
                        )

                def oden_block(bk, at2):
                    # o + denominator for pairs (2bk, 2bk+1) from masked attn
                    # at2 cols = (r, pp, hg, i2w)
                    p0 = 2 * bk
                    for hg in range(2):
                        od = psp.tile([128, 512], f32, tag="od", bufs=4,
                                      name="pod")
                        for q in range(2):
                            for r in range(4):
                                h = 4 * hg + r
                                nc.tensor.matmul(
                                    od[r * 32 : (r + 1) * 32,
                                       q * 128 : (q + 1) * 128],
                                    lhsT=vt[:, (p0 + q) * 256 + h * 32 :
                                            (p0 + q) * 256 + (h + 1) * 32],
                                    rhs=at2[:, r * 512 + q * 256 + hg * 128 :
                                            r * 512 + q * 256 + (hg + 1) * 128],
                                    start=True, stop=True,
                                    tile_position=(0, r * 32),
                                )
                        at4 = at2.rearrange("p (r q x) -> p r q x",
                                            r=4, q=2, x=256)
                        for r in range(4):
                            nc.tensor.matmul(
                                od[r * 32 : (r + 1) * 32, 256:512],
                                lhsT=ones_sb[:],
                                rhs=at4[:, r, :, hg * 128 : (hg + 1) * 128],
                                start=True, stop=True,
                                tile_position=(0, r * 32),
                            )
                        rc = rcpp.tile([128, 256], f32, tag="rc", name="rc")
                        nc.vector.reciprocal(rc[:], od[:, 256:512])
                        nc.vector.tensor_tensor(
                            oa[(hg, bk // 2)][:, (bk % 2) * 256 :
                                              (bk % 2 + 1) * 256],
                            od[:, 0:256], rc[:], MULT,
                        )

                def out_proj(nt):
                    # output projection for spatial block nt (512 cols)
                    for mc in range(2):
                        ps = psp.tile([128, 512], f32, tag="od", bufs=4,
                                      name="pod")
                        for kc in range(2):
                            nc.tensor.matmul(
                                ps[:],
                                lhsT=wo_sb[kc][:, mc * 128 : (mc + 1) * 128],
                                rhs=oa[(kc, nt)][:],
                                start=(kc == 0),
                                stop=(kc == 1),
                            )
                        yt = ysbp.tile([128, 512], f32, tag="ysb", name="ysb")
                        psv = ps.rearrange(
                            "p (ww ph pw) -> p ww ph pw", ww=NW, ph=P, pw=P
                        )
                        nc.vector.tensor_scalar_add(
                            _raster_scatter(yt), psv, bout_sb[mc][:],
                        )
                        nc.sync.dma_start(
                            y_d[b, mc * 128 : (mc + 1) * 128,
                                nt * 512 : (nt + 1) * 512],
                            yt[:],
                        )

                at_prev = None
                for bk in range(16):
                    at2 = attp.tile([128, 2048], bf16, tag="at2", name="at2")
                    # scores for both pairs of the block: bank = head strip r
                    # (single row-group tile_position per PSUM bank), cols
                    # within a bank = (pp, hg, i2w)
                    scA = psp.tile([128, 1024], f32, tag="scA", bufs=1,
                                   name="psc")
                    scB = psp.tile([128, 1024], f32, tag="scB", bufs=1,
                                   name="psc")
                    for pp in range(2):
                        p = 2 * bk + pp
                        nt2, col = p // 8, (p % 8) * 128
                        for hg in range(2):
                            for r in range(4):
                                sc = scA if r < 2 else scB
                                nc.tensor.matmul(
                                    sc[:, (r % 2) * 512 + pp * 256 + hg * 128 :
                                       (r % 2) * 512 + pp * 256 + (hg + 1) * 128],
                                    lhsT=qk_t[(2 + hg, nt2)][r * 32 : (r + 1) * 32,
                                                             col : col + 128],
                                    rhs=qk_t[(hg, nt2)][r * 32 : (r + 1) * 32,
                                                        col : col + 128],
                                    start=True, stop=True,
                                    tile_position=(r * 32, 0),
                                )
                    for r in range(4):
                        sc = scA if r < 2 else scB
                        nc.scalar.activation(
                            at2[:, r * 512 : (r + 1) * 512],
                            sc[:, (r % 2) * 512 : (r % 2 + 1) * 512], EXP
                        )
                    # bf16 multiply masks cross-window quadrants and applies
                    # exp(bias)
                    nc.vector.tensor_tensor(at2[:], at2[:], eb_sb[:], MULT)
                    if at_prev is not None:
                        oden_block(bk - 1, at_prev)
                        if bk % 2 == 0:
                            out_proj((bk - 2) // 2)
                    at_prev = at2
                    if bk == 10 and b + 1 < B_LOC:
                        xw_next = load_image(b + 1)
                oden_block(15, at_prev)
                out_proj(7)

    nc.compile()
    return nc


def _prep_consts(w_proj, position, w_out, b_out):
    import ml_dtypes
    bf16 = ml_dtypes.bfloat16
    scale = 1.0 / np.sqrt(np.float32(D))
    w_qkT = np.ascontiguousarray(w_proj[:512].T).astype(np.float32)
    w_qkT[:, :256] *= scale
    w_qkT = w_qkT.astype(bf16)
    w_vT = np.ascontiguousarray(w_proj[512:].T).astype(bf16)
    w_outT = np.ascontiguousarray(w_out.T).astype(bf16)
    bias = _rel_bias_np(position.astype(np.float32))  # [h, i(query), j(key)]
    # eb_full[wj*64+j, r*512 + pp*256 + hg*128 + wi*64 + i] =
    #   exp(bias[4hg+r][i, j]) if wi == wj else 0   (replicated over pp)
    eb = np.zeros((128, 2048), np.float32)
    for r in range(4):
        for pp in range(2):
            for hg in range(2):
                ebT = np.exp(bias[4 * hg + r].T)  # [j, i]
                base = r * 512 + pp * 256 + hg * 128
                for wj in range(2):
                    eb[wj * 64 : (wj + 1) * 64,
                       base + wj * 64 : base + (wj + 1) * 64] = ebT
    eb_full = eb.astype(bf16)
    ones_c = np.ones((128, 32), bf16)
    b_out2 = np.ascontiguousarray(
        b_out.astype(np.float32).reshape(2, 128, 1)
    )
    return {
        "w_qkT": w_qkT,
        "w_vT": w_vT,
        "w_outT": w_outT,
        "eb_full": eb_full,
        "ones_c": ones_c,
        "b_out2": b_out2,
    }


def kernel(x, w_proj, position, w_out, b_out):
    import ml_dtypes
    from concourse.bass_utils import run_bass_kernel_spmd

    if "nc" not in _CACHE:
        _CACHE["nc"] = _build()
    nc = _CACHE["nc"]

    consts = _prep_consts(w_proj, position, w_out, b_out)
    x = np.asarray(x, np.float32).reshape(B, C, HW).astype(ml_dtypes.bfloat16)
    in_maps = []
    for i in range(NCORES):
        m = dict(consts)
        m["x_sh"] = np.ascontiguousarray(x[i * B_LOC : (i + 1) * B_LOC])
        in_maps.append(m)

    res = run_bass_kernel_spmd(nc, in_maps, core_ids=list(range(NCORES)))
    out = np.concatenate([res.results[i]["y_sh"] for i in range(NCORES)], axis=0)
    return out.reshape(B, C, H, W)
